# revision 22
# baseline (speedup 1.0000x reference)
"""LIF spike (vanilla) Trainium2 kernel — time-bit-packed u8 output, PE pack.

Reference recurrence over leading time dim T (per element):
    u_t = TAU * u_{t-1} * (1 - o_{t-1}) + x_t
    o_t = (u_t - VTH > 0) ? 1.0 : 0.0

Device-side structure per (chunk, t):
    S1 (DVE):  u_t = TAU * select(u_{t-1} <= VTH, u_{t-1}, 0) + x_t
        -- ONE custom DVE op (LIF_GATED_DECAY_ADD_ANT), registered below.
           select(u < nextafter(VTH), u, 0) == u * (u <= VTH) exactly, and
           TAU = 0.5 is a power of two so TAU*u is exact: u_t matches the
           fp32 jax reference bit-for-bit.
    S2 (ACT):  s_t = Sign(u_t - VTH) in {-1, 0, +1}, bf16
    S3 (PE):   p += diag(2^(T-1-t)) @ s_t, accumulated in PSUM (fp32, exact)

After t=T-1, p = sum_t s_t * 2^(T-1-t) in [-255, 255]; one ACT copy maps
it to u8 via (p + 255)/2 (scale=0.5, bias=127.5 — exact for odd-integer p)
and one DMA per chunk stores it: output traffic drops 16 MiB -> 1 MiB per
core. Host decode: unpackbits, bit (T-1-t) = spike_t.

Chunk sizes taper (2048 x3, 1024, 512 x2): the recurrence over T is a
serial chain per chunk, so the final chunk — whose x tiles arrive last,
when DMA is already saturated — is kept small to shorten the drain tail.
Each chunk-size class gets its own DRAM tensor so every [P, F] tile is a
contiguous block (strided rows cost ~12% DMA throughput, measured).

Sharding: pure data parallel over batch dim B=64 -> 8 cores x 8 batches.
Per core: 32 MiB in (f32) + 1 MiB out (u8) HBM traffic.
"""

import numpy as np

T = 8
B = 64
C = 128
H = 32
W = 32
NCORES = 8
BS = B // NCORES            # batches per core
N = BS * C * H * W          # 1,048,576 elements per time step per core
P = 128                     # SBUF partitions
FTOT = N // P               # 8192 free-dim elements per partition per t
# tapered tail: (dram tensor name, count, tile free-dim)
CHUNK_CLASSES = (("x2048", 3, 2048), ("x1024", 1, 1024), ("x512", 2, 512))
assert sum(n * f for _, n, f in CHUNK_CLASSES) == FTOT
TAU = 0.5
VTH = 0.99999
VTH_PLUS = float(np.nextafter(np.float32(VTH), np.float32(np.inf)))
PSUM_BANK_F = 512           # f32 elements per partition per PSUM bank


def _digit_weights():
    """[T, 128, 128] bf16: W_t = diag(2^(T-1-t)) — PE pack weights."""
    import ml_dtypes

    w = np.zeros((T, P, P), np.float32)
    for t in range(T):
        w[t] = np.eye(P, dtype=np.float32) * float(2 ** (T - 1 - t))
    return w.astype(ml_dtypes.bfloat16)


def _register_lif_op():
    """Register the fused LIF decay custom DVE op (idempotent).

    out = select(in0 < s0, in0, 0) * s1 + in1
    """
    from concourse import dve_ops
    from concourse.dve_spec import C0, C1, Spec, Src0, Src1, Zero, select
    from concourse.dve_spec import _has_src1, lower
    from concourse.dve_uop import DveOpSpec

    name = "LIF_GATED_DECAY_ADD_ANT"
    for op in dve_ops.OPS:
        if op.name == name:
            return op
    spec = Spec(
        body=select(Src0 < C0, Src0, Zero) * C1 + Src1,
        reference=lambda in0, in1, s0, s1, imm2: (
            np.where(in0 < s0, in0, np.float32(0.0)).astype(np.float32)
            * np.float32(s1)
            + in1
        ).astype(np.float32),
    )
    row = dve_ops._CUSTOM_DVE_ROW_BASE + len(dve_ops.OPS)
    assert row < 0x20, "custom-DVE opcode rows exhausted"
    shas = {}
    for ver in ("v3", "v4"):
        tmp = DveOpSpec(
            name=name, opcode=row, uops=lower(spec, ver=ver),
            rd1_en=_has_src1(spec),
        )
        shas[ver] = tmp.sha(ver)
    op = dve_ops.DveOp(name, spec, subdim=False, uops_sha=shas)
    dve_ops.OPS.append(op)
    dve_ops.CUSTOM_DVE_SPECS[name] = spec
    dve_ops._SUB_OPCODE_FOR_NAME[name] = row
    return op


def _build(nt=T, xb=6, ub=3, sb=4, ob=2):
    import concourse.bacc as bacc
    import concourse.bass as bass
    import concourse.mybir as mybir
    import concourse.tile as tile

    lif_op = _register_lif_op()

    f32 = mybir.dt.float32
    bf16 = mybir.dt.bfloat16
    u8 = mybir.dt.uint8
    act = mybir.ActivationFunctionType
    nc = bacc.Bacc("TRN2", target_bir_lowering=False)
    xts_dram = {
        name: nc.dram_tensor(name, [nt, cnt, P, fi], f32, kind="ExternalInput")
        for name, cnt, fi in CHUNK_CLASSES
    }
    ots_dram = {
        name: nc.dram_tensor("o" + name[1:], [cnt, P, fi], u8,
                             kind="ExternalOutput")
        for name, cnt, fi in CHUNK_CLASSES
    }
    w = nc.dram_tensor("w", [nt, P, P], bf16, kind="ExternalInput")
    with tile.TileContext(nc) as tc:
        with (
            tc.tile_pool(name="const", bufs=1) as constp,
            tc.tile_pool(name="xp", bufs=xb) as xp,
            tc.tile_pool(name="up", bufs=ub) as up,
            tc.tile_pool(name="sp", bufs=sb) as sp,
            tc.tile_pool(name="op", bufs=ob) as op_,
            tc.tile_pool(name="pp", bufs=2, space=bass.MemorySpace.PSUM) as pp,
        ):
            nvth = constp.tile([P, 1], f32)
            nc.vector.memset(nvth[:], -VTH)
            wsb = constp.tile([P, nt, P], bf16)
            for t in range(nt):
                nc.sync.dma_start(wsb[:, t, :], w[t])
            for name, cnt, fi in CHUNK_CLASSES:
                xd, od = xts_dram[name], ots_dram[name]
                for i in range(cnt):
                    p = pp.tile([P, fi], f32, name="p")
                    u = None
                    for t in range(nt):
                        xt = xp.tile([P, fi], f32, name="xt")
                        nc.sync.dma_start(xt[:], xd[t, i])
                        if t == 0:
                            u = xt
                        else:
                            un = up.tile([P, fi], f32, name="un")
                            nc.vector._custom_dve(
                                lif_op, out=un[:], in0=u[:], in1=xt[:],
                                s0=VTH_PLUS, s1=TAU,
                            )
                            u = un
                        st = sp.tile([P, fi], bf16, name="st")
                        nc.scalar.activation(
                            st[:], u[:], act.Sign, bias=nvth[:], scale=1.0,
                        )
                        # one Matmult may only target a single PSUM bank
                        # (512 f32 per partition): split across banks.
                        for j in range(fi // PSUM_BANK_F):
                            sl = slice(j * PSUM_BANK_F, (j + 1) * PSUM_BANK_F)
                            nc.tensor.matmul(
                                p[:, sl], wsb[:, t, :], st[:, sl],
                                start=(t == 0), stop=(t == nt - 1),
                            )
                    ot = op_.tile([P, fi], u8, name="ot")
                    # u8 spike byte: (p + 255) / 2, exact for odd-integer p
                    nc.scalar.activation(
                        ot[:], p[:], act.Copy, bias=127.5, scale=0.5,
                    )
                    nc.sync.dma_start(od[i], ot[:])
    nc.finalize()
    return nc


def _in_maps(x):
    wdig = _digit_weights()
    in_maps = []
    for c in range(NCORES):
        s = np.ascontiguousarray(x[:, c * BS : (c + 1) * BS]).reshape(T, N)
        m = {"w": wdig}
        base = 0
        for name, cnt, fi in CHUNK_CLASSES:
            seg = s[:, base : base + cnt * P * fi]
            m[name] = np.ascontiguousarray(seg.reshape(T, cnt, P, fi))
            base += cnt * P * fi
        in_maps.append(m)
    return in_maps


def kernel(x):
    x = np.ascontiguousarray(np.asarray(x, dtype=np.float32))
    assert x.shape == (T, B, C, H, W), x.shape
    from concourse.bass_utils import run_bass_kernel_spmd

    nc = _build()
    res = run_bass_kernel_spmd(nc, _in_maps(x), core_ids=list(range(NCORES)))
    out = np.empty((T, B, C, H, W), np.float32)
    for i, r in enumerate(res.results):
        out[:, i * BS : (i + 1) * BS] = _decode(r)
    return out


def _decode(r):
    """Per-core result dict -> f32 spike train [T, BS, C, H, W].

    byte = (p + 255)/2 with p = sum_t s_t * 2^(T-1-t), s_t in {-1,+1}:
    bit (T-1-t) = spike_t.
    """
    s = np.concatenate(
        [np.asarray(r["o" + name[1:]]).reshape(-1) for name, _, _ in CHUNK_CLASSES]
    )                                                          # [N] u8
    bits = np.unpackbits(s[:, None], axis=1, bitorder="big")   # [N, T]
    return bits.T.astype(np.float32).reshape(T, BS, C, H, W)
